# revision 52
# baseline (speedup 1.0000x reference)
"""Trainium2 Bass kernel: AttentionAggregator (GNN message passing).

Reference math per branch (user / item):
    cat  = concat_k [ tabA[adjA[:,k]] | tabB[adjB[:,k]] ]      # [NJ, 256]
    S    = (q @ q.T) / sqrt(D) + 1                             # [NJ, NJ]
    agg  = softmax(S, axis=-1) @ cat                           # [NJ, 256]
    out  = relu(agg @ W)                                       # [NJ, 64]

Key refactorings used here:
  * (softmax(S) @ cat) @ W == softmax(S) @ (cat @ W): precompute VW = cat @ W
    ([NJ, 64]) so the big GEMM has 65 columns instead of 256.
  * Scores are bounded (|S| <~ 15) so softmax needs no max subtraction:
    row = (exp(S) @ [VW | 1]); out = relu(row[:64]) / row[64].
  * relu(x / s) == relu(x) / s for s > 0, so normalization happens last.

Sharding: 8 cores, row-parallel, no collectives. Cores 0-3 take 2048-row
slices of the user branch, cores 4-7 of the item branch. Both branches have
identical shapes (tabA = review[16384], tabB = item/user[8192]) so one SPMD
program serves all 8 cores with different data.

Per-core dataflow:
  phase A (neighbor gather):
    indirect-DMA gather of bf16 table rows -> cat [NJ, 256] in SBUF
    -> DRAM bounce -> XBAR transpose-load -> catT [256, NJ]
    -> PE: VW = catT.T @ W per 128-row j-tile -> VW1 [NJ, 65] bf16 in SBUF
  main loop over (m-block of 512) x (j-tile of 128):
    PE:  S^T tile  = qT[:, jtile].T @ qmT[:, mblock]   (K=32, fp32 PSUM)
    ACT: E = exp(S^T / sqrt(D) + 1)  -> bf16 SBUF
    PE:  out_psum[65, mblock] += VW1[jtile].T @ E      (accumulate over j)
  epilogue per m-block:
    copy out_psum -> SBUF, PE-transpose 128-col chunks -> [128, 65],
    DVE: out = max(Z * recip(rowsum), 0) -> DMA to DRAM.
"""

import os
import sys

sys.path.insert(0, "/opt/trn_rl_repo")
os.environ.setdefault("MYCRO_LOCAL_CACHE", "1")

import numpy as np

import concourse.bass as bass
import concourse.bacc as bacc
import concourse.mybir as mybir
import concourse.tile as tile
from concourse.masks import make_identity

try:  # ml_dtypes ships with jax
    import ml_dtypes

    BF16_NP = ml_dtypes.bfloat16
except ImportError:  # pragma: no cover
    BF16_NP = None

P = 128


class Cfg:
    def __init__(self, NJ=8192, M=2048, NA=16384, NB=8192, D=32, K=4, OUT=64,
                 MBLK=1024, CH=1024, GSH=4):
        self.NJ = NJ      # attention length (rows of the branch)
        self.M = M        # rows this core owns
        self.NA = NA      # table A rows (review_vecs)
        self.NB = NB      # table B rows (item/user vecs)
        self.D = D        # embedding dim (contraction for scores)
        self.K = K        # neighbors per adjacency list
        self.OUT = OUT    # output dim
        self.MBLK = MBLK  # m-block width per exp/psum tile
        self.CH = CH      # gather chunk (j rows per gather round)
        self.GSH = GSH    # cores per branch group sharing the gather
        self.F = 2 * K * D            # cat width (256)
        self.JT = NJ // P             # j-tiles
        self.NMB = M // MBLK          # m-blocks
        self.CT = CH // P             # j-tiles per gather chunk
        self.JSH = NJ // GSH          # j rows gathered by this core
        self.SHT = self.JSH // P      # j-tiles in this core's shard
        self.NCH = self.JSH // CH     # gather chunks (local)
        self.FH = self.F // P         # 128-row halves of cat width (2)
        assert NJ % P == 0 and M % MBLK == 0 and CH % P == 0
        assert self.JSH % CH == 0 and self.JSH % 16 == 0
        assert self.F % P == 0


def build_nc(cfg: Cfg, debug: bool = False) -> bass.Bass:
    NJ, M, NA, NB, D, K, OUT = cfg.NJ, cfg.M, cfg.NA, cfg.NB, cfg.D, cfg.K, cfg.OUT
    MBLK, CH, F, JT, NMB, CT, NCH, FH = (cfg.MBLK, cfg.CH, cfg.F, cfg.JT,
                                         cfg.NMB, cfg.CT, cfg.NCH, cfg.FH)
    GSH, JSH, SHT = cfg.GSH, cfg.JSH, cfg.SHT
    O1 = OUT + 1
    bf16 = mybir.dt.bfloat16
    fp32 = mybir.dt.float32

    nc = bacc.Bacc(num_devices=2 * GSH, num_swdge_queues=4)
    groups = [list(range(GSH)), list(range(GSH, 2 * GSH))]

    i16 = mybir.dt.int16
    NPAIR = 2 * K  # (table, k) gather streams: A k0..k3 then B k0..k3
    qT = nc.declare_dram_parameter("qT", [D, NJ], bf16, isOutput=False)
    qmT = nc.declare_dram_parameter("qmT", [D, M], bf16, isOutput=False)
    # tables padded to 128 elems/row (256B) for dma_gather's elem constraint
    tabA = nc.declare_dram_parameter("tabA", [NA, P], bf16, isOutput=False)
    tabB = nc.declare_dram_parameter("tabB", [NB, P], bf16, isOutput=False)
    # int16 indices for THIS core's j-shard, 16-partition-wrapped and
    # replicated across Q7 cores: idx[pair, p, s] = adj[js0 + s*16 + p%16, k]
    idx = nc.declare_dram_parameter("idx", [NPAIR, P, JSH // 16], i16,
                                    isOutput=False)
    Wp = nc.declare_dram_parameter("Wp", [F, OUT], bf16, isOutput=False)
    out = nc.declare_dram_parameter("out", [M, OUT], fp32, isOutput=True)

    # per-chunk DRAM bounce buffers for the cat -> catT transpose
    cat_dram = [nc.dram_tensor(f"cat_dram{c}", [CH, F], bf16) for c in range(NCH)]
    # VW shard exchange buffers (chunked AllGather over the branch group)
    vw_shard_dram = nc.dram_tensor("vw_shard", [SHT, P, O1], bf16)
    vw_full_dram = [nc.dram_tensor(f"vw_full{c}", [GSH * CT, P, O1], bf16)
                    for c in range(NCH)]
    # j-tile processing order: chunk-c tiles of every rank come before
    # chunk-c+1 tiles, matching chunked-AllGather availability.
    t_order = [r * SHT + c * CT + i
               for c in range(NCH) for r in range(GSH) for i in range(CT)]
    assert sorted(t_order) == list(range(JT))
    if debug:
        cat_dbg = nc.declare_dram_parameter("cat_dbg", [NCH, CH, F], bf16,
                                            isOutput=True)
        vw_dbg = nc.declare_dram_parameter("vw_dbg", [P, JT, OUT + 1], bf16,
                                           isOutput=True)
        catT_dbg = nc.declare_dram_parameter("catT_dbg", [NCH, P, FH, CH], bf16,
                                             isOutput=True)

    with tile.TileContext(nc) as tc:
        with (
            tc.tile_pool(name="const", bufs=1) as const_pool,
            tc.tile_pool(name="gat", bufs=4) as gat_pool,
            tc.tile_pool(name="catT", bufs=3) as catT_pool,
            tc.tile_pool(name="vw", bufs=2 * NCH) as vw_pool,
            tc.tile_pool(name="exp", bufs=24) as exp_pool,
            tc.tile_pool(name="ocopy", bufs=2) as oc_pool,
            tc.tile_pool(name="rec", bufs=2) as rec_pool,
            tc.tile_pool(name="ostage", bufs=3) as ost_pool,
            tc.tile_pool(name="sps", bufs=2, space="PSUM") as sps_pool,
            tc.tile_pool(name="ops", bufs=1, space="PSUM") as ops_pool,
            tc.tile_pool(name="scrps", bufs=2, space="PSUM") as scr_pool,
        ):
            # ---- constants / persistent SBUF tensors -----------------------
            qT_sb = const_pool.tile([P, NJ], bf16, tag="qT_sb")
            nc.sync.dma_start(out=qT_sb[0:D, :], in_=qT[:, :])
            qmT_sb = const_pool.tile([P, M], bf16, tag="qmT_sb")
            nc.sync.dma_start(out=qmT_sb[0:D, :], in_=qmT[:, :])

            W_sb = const_pool.tile([P, FH, OUT], bf16, tag="W_sb")
            for h in range(FH):
                nc.sync.dma_start(out=W_sb[:, h, :], in_=Wp[h * P:(h + 1) * P, :])

            idx_sb = const_pool.tile([P, NPAIR, JSH // 16], i16, tag="idx_sb")
            for pr in range(NPAIR):
                nc.sync.dma_start(out=idx_sb[:, pr, :], in_=idx[pr, :, :])

            ident = const_pool.tile([P, P], fp32, tag="ident")
            make_identity(nc, ident[:])

            bias1 = const_pool.tile([P, 1], fp32, tag="bias1")
            nc.gpsimd.memset(bias1[:], 1.0)

            # Warm-up Exp so the ACT table-set pseudo-load lands on an
            # instruction with few sync waits (walrus limit: 2 per inst),
            # not on the first pipelined exp of the main loop.
            warm = const_pool.tile([P, 1], fp32, tag="warm")
            nc.scalar.activation(
                out=warm[:], in_=bias1[:],
                func=mybir.ActivationFunctionType.Exp,
                bias=bias1[:, 0:1], scale=1.0)

            # ---- phase A: gather this core's j-shard, transpose, VW --------
            qnum = 0
            for c in range(NCH):
                # gather each (table, k) stream; row j -> partition j%128,
                # slot j//128, then bounce the real 32 elems into cat_dram
                # at f-column k*64 (+32 for the B table).
                cat_cols = cat_dram[c][:, :].rearrange("(t p) f -> p t f", p=P)
                for pr in range(NPAIR):
                    tab_src = tabA if pr < K else tabB
                    k = pr % K
                    col0 = k * 2 * D + (0 if pr < K else D)
                    gat = gat_pool.tile([P, CT, P], bf16, tag="gat")
                    nc.gpsimd.dma_gather(
                        gat[:],
                        tab_src[:, :],
                        idx_sb[:, pr, c * (CH // 16):(c + 1) * (CH // 16)],
                        CH,
                        CH,
                        P,
                        queue_num=qnum % 4,
                    )
                    qnum += 1
                    nc.sync.dma_start(
                        out=cat_cols[:, :, col0:col0 + D],
                        in_=gat[:, :, 0:D],
                    )
                # XBAR transpose-load: catT[f, j] for this chunk
                catT_sb = catT_pool.tile([P, FH, CH], bf16, tag="catT_sb")
                for h in range(FH):
                    nc.sync.dma_start(
                        out=catT_sb[:, h, :],
                        in_=cat_dram[c][:, h * P:(h + 1) * P],
                        transpose=True,
                    )
                # VW for each j-tile of this chunk; the matmuls are gated to
                # their realistic ready time (PE-only) so they don't block
                # main-loop S matmuls behind them in the PE stream.
                vw1 = vw_pool.tile([P, CT, O1], bf16, tag="vw1")
                nc.vector.memset(vw1[:, :, OUT:O1], 1.0)
                for tl in range(CT):
                    vps = scr_pool.tile([P, O1], fp32, tag="scr")
                    for h in range(FH):
                        nc.tensor.matmul(
                            out=vps[:, 0:OUT],
                            lhsT=catT_sb[:, h, tl * P:(tl + 1) * P],
                            rhs=W_sb[:, h, :],
                            start=(h == 0),
                            stop=(h == FH - 1),
                        )
                    nc.vector.tensor_copy(out=vw1[:, tl, 0:OUT],
                                          in_=vps[:, 0:OUT])
                # ship this chunk's VW tiles to the exchange buffer
                nc.sync.dma_start(
                    out=vw_shard_dram[c * CT:(c + 1) * CT, :, :].rearrange(
                        "t p c -> p t c"),
                    in_=vw1[:],
                )
                if debug:
                    dbg_sb = gat_pool.tile([P, CT, F], bf16, tag="dbg_sb")
                    nc.sync.dma_start(
                        out=dbg_sb[:],
                        in_=cat_dram[c][:, :].rearrange("(t p) f -> p t f", p=P))
                    nc.sync.dma_start(out=cat_dbg[c, :, :].rearrange(
                        "(t p) f -> p t f", p=P), in_=dbg_sb[:])
                    nc.sync.dma_start(out=catT_dbg[c, :, :, :], in_=catT_sb[:])

            # ---- exchange VW shards within the branch group ---------------
            # one AllGather per chunk; separate SBUF tiles per chunk so the
            # main loop's G matmuls only wait on the chunk they consume.
            vw_sbs = []
            for c in range(NCH):
                nc.gpsimd.collective_compute(
                    "AllGather",
                    mybir.AluOpType.bypass,
                    replica_groups=groups,
                    ins=[vw_shard_dram[c * CT:(c + 1) * CT, :, :]],
                    outs=[vw_full_dram[c][:, :, :]],
                )
                vw_c = vw_pool.tile([P, GSH * CT, O1], bf16, tag="vw_c")
                nc.sync.dma_start(
                    out=vw_c[:],
                    in_=vw_full_dram[c][:, :, :].rearrange("t p c -> p t c"),
                )
                vw_sbs.append(vw_c)
            if debug:
                for c in range(NCH):
                    nc.sync.dma_start(
                        out=vw_dbg[:, c * GSH * CT:(c + 1) * GSH * CT, :],
                        in_=vw_sbs[c][:])

            # ---- main loop -------------------------------------------------
            inv_sqrt_d = 1.0 / float(np.sqrt(D))
            NHALF = MBLK // 512 if MBLK >= 512 else 1
            HB = MBLK // NHALF  # matmul moving width (<= 512)
            for mb in range(NMB):
                out_ps = ops_pool.tile([O1, MBLK], fp32, tag="out_ps")
                for s in range(JT):
                    t = t_order[s]
                    s_ps = sps_pool.tile([P, MBLK], fp32, tag="s_ps")
                    for h in range(NHALF):
                        nc.tensor.matmul(
                            out=s_ps[:, h * HB:(h + 1) * HB],
                            lhsT=qT_sb[0:D, t * P:(t + 1) * P],
                            rhs=qmT_sb[0:D,
                                       mb * MBLK + h * HB:mb * MBLK + (h + 1) * HB],
                            start=True,
                            stop=True,
                        )
                    e_sb = exp_pool.tile([P, MBLK], bf16, tag="e_sb")
                    nc.scalar.activation(
                        out=e_sb[:],
                        in_=s_ps[:],
                        func=mybir.ActivationFunctionType.Exp,
                        bias=bias1[:, 0:1],
                        scale=inv_sqrt_d,
                    )
                    for h in range(NHALF):
                        nc.tensor.matmul(
                            out=out_ps[:, h * HB:(h + 1) * HB],
                            lhsT=vw_sbs[s // (GSH * CT)][:, s % (GSH * CT), :],
                            rhs=e_sb[:, h * HB:(h + 1) * HB],
                            start=(s == 0),
                            stop=(s == JT - 1),
                            skip_group_check=True,
                        )
                # epilogue: transpose + normalize + relu + store
                oc_sb = oc_pool.tile([O1, MBLK], fp32, tag="oc_sb")
                nc.vector.tensor_copy(out=oc_sb[:], in_=out_ps[:])
                o_sb = ost_pool.tile([P, MBLK // P, OUT], fp32, tag="o_sb")
                for q in range(MBLK // P):
                    t_ps = scr_pool.tile([P, O1], fp32, tag="scr")
                    nc.tensor.transpose(
                        out=t_ps[:],
                        in_=oc_sb[:, q * P:(q + 1) * P],
                        identity=ident[0:O1, 0:O1],
                    )
                    rec = rec_pool.tile([P, 1], fp32, tag="rec")
                    nc.vector.reciprocal(out=rec[:], in_=t_ps[:, OUT:O1])
                    nc.vector.tensor_scalar(
                        out=o_sb[:, q, :],
                        in0=t_ps[:, 0:OUT],
                        scalar1=rec[:, 0:1],
                        scalar2=0.0,
                        op0=mybir.AluOpType.mult,
                        op1=mybir.AluOpType.max,
                    )
                nc.sync.dma_start(
                    out=out[mb * MBLK:(mb + 1) * MBLK, :].rearrange(
                        "(q p) c -> p q c", p=P),
                    in_=o_sb[:],
                )

    nc.finalize()
    return nc


# --------------------------------------------------------------------------
# host side
# --------------------------------------------------------------------------

def _bf16(a: np.ndarray) -> np.ndarray:
    return np.ascontiguousarray(a.astype(BF16_NP))


def _pad_table(t: np.ndarray) -> np.ndarray:
    # [N, D] -> [N, 128] bf16, zeros in the pad
    out = np.zeros((t.shape[0], P), BF16_NP)
    out[:, : t.shape[1]] = np.asarray(t).astype(BF16_NP)
    return out


def _wrap_idx(arr: np.ndarray) -> np.ndarray:
    # [NJ] -> [128, NJ//16] int16: idx position i = s*16 + p%16 at [p, s],
    # replicated across the eight 16-partition Q7 groups
    w = arr.astype(np.int16).reshape(-1, 16).T  # [16, NJ//16]
    return np.ascontiguousarray(np.tile(w, (P // 16, 1)))


def _make_idx(adjA: np.ndarray, adjB: np.ndarray, K: int) -> np.ndarray:
    streams = [_wrap_idx(adjA[:, k]) for k in range(K)]
    streams += [_wrap_idx(adjB[:, k]) for k in range(K)]
    return np.ascontiguousarray(np.stack(streams, axis=0))


def _make_in_maps(cfg: Cfg, review_vecs, user_vecs, item_vecs,
                  adj_ur, adj_ri, adj_ir, adj_ru,
                  user_neigh_W, item_neigh_W, n_cores=8):
    half = n_cores // 2
    rT = _pad_table(np.asarray(review_vecs))
    uT = _bf16(np.asarray(user_vecs).T)
    iT = _bf16(np.asarray(item_vecs).T)
    uW = _bf16(np.asarray(user_neigh_W))
    iW = _bf16(np.asarray(item_neigh_W))
    idx_u = _make_idx(np.asarray(adj_ur), np.asarray(adj_ri), cfg.K)
    idx_i = _make_idx(np.asarray(adj_ir), np.asarray(adj_ru), cfg.K)
    SW = cfg.JSH // 16  # wrapped-index columns per shard
    item_tab = _pad_table(np.asarray(item_vecs))
    user_tab = _pad_table(np.asarray(user_vecs))

    in_maps = []
    for core in range(n_cores):
        if core < half:  # user branch
            qTb, tab_b, ix, w = uT, item_tab, idx_u, uW
        else:  # item branch
            qTb, tab_b, ix, w = iT, user_tab, idx_i, iW
        s = (core % half) * cfg.M
        sh = core % cfg.GSH
        in_maps.append({
            "qT": qTb,
            "qmT": np.ascontiguousarray(qTb[:, s:s + cfg.M]),
            "tabA": rT,
            "tabB": tab_b,
            "idx": np.ascontiguousarray(ix[:, :, sh * SW:(sh + 1) * SW]),
            "Wp": w,
        })
    return in_maps


_BUILT = {}


def _get_nc(cfg: Cfg) -> bass.Bass:
    key = tuple(sorted(cfg.__dict__.items()))
    if key not in _BUILT:
        _BUILT[key] = build_nc(cfg)
    return _BUILT[key]


def kernel(review_vecs, user_vecs, item_vecs, adj_ur, adj_ri, adj_ir, adj_ru,
           user_neigh_W, item_neigh_W, _trace=False):
    from concourse.bass_utils import run_bass_kernel_spmd

    n_cores = 8
    Nu = np.asarray(user_vecs).shape[0]
    Ni = np.asarray(item_vecs).shape[0]
    cfg = Cfg(NJ=Nu, M=Nu // (n_cores // 2),
              NA=np.asarray(review_vecs).shape[0], NB=Ni)
    nc = _get_nc(cfg)
    in_maps = _make_in_maps(cfg, review_vecs, user_vecs, item_vecs,
                            adj_ur, adj_ri, adj_ir, adj_ru,
                            user_neigh_W, item_neigh_W, n_cores)
    res = run_bass_kernel_spmd(nc, in_maps, core_ids=list(range(n_cores)),
                               trace=_trace)
    outs = [np.asarray(res.results[i]["out"], dtype=np.float32)
            for i in range(n_cores)]
    user_out = np.concatenate(outs[: n_cores // 2], axis=0)
    item_out = np.concatenate(outs[n_cores // 2:], axis=0)
    if _trace:
        return (user_out, item_out), res
    return user_out, item_out
